# revision 50
# baseline (speedup 1.0000x reference)
"""Trainium2 Bass kernel for 16-head causal MHA (B=2, S=2048, D=1024), fp32.

Sharding (8 cores): batch x head-group. Core c handles batch c//4 and heads
4*(c%4)..4*(c%4)+3 (D columns 256*(c%4) .. +256). QKV weights column-sharded,
Wo row-sharded (Megatron). Per-core partial outputs are summed on the host
(the row-parallel all-reduce), bo added once per batch.

x is transposed on the HOST (xt = x[b].T, [D, S]) so the kernel needs no PE
transposes at all: projections read xt directly as [D-part, tok] tiles.

Per-core dataflow (all matmuls in float32r: ~12-bit mantissa, full speed):
  phase 1 (per 512-token chunk): qT/kT = W.T @ xt ([head-dim part, tok free])
           + bias; v computed in NATURAL layout per 128-token block
           (xt-block stationary, Wv moving: out [tok part, 256]), written
           into v_all slots with a ones column appended (softmax denom).
  phase 2 (per q-superblock i, head-pair p, k-block j<=4i+3):
           S_T[tk,tq] = k @ qT (2 heads packed in PE row groups);
           expS = exp(S_T) straight to f32r (no max subtraction; scores O(6));
           diagonal-band tiles: triangular mask-mul (DVE); the last diag
           block (r=3) runs at free=256 (not 128) to dodge the fp32r
           small-free 4x penalty, with a [0|tri] mask.
           ctxT[hd+1, tq] += [v|1].T @ expS  (ones row = softmax denominator);
           evict via ACT/DVE copies (GPSIMD cannot touch PSUM), normalize
           via K=1 broadcast matmul + DVE reciprocal + Pool mul.
  phase 3: out[tq, :] = ctxT_norm.T @ Wo_slice (partial), DMA out (SP queue,
           HW DGE; Pool's software DGE costs ~1us of Pool time per store).

Emission interleaves phase1(c+1) / phase3(c-1) units into phase2(c)'s
j-loop so PE fills its ACT-wait gaps with projection work, and is
software-pipelined across the loop_n hardware loop: the body's last
superblock carries the NEXT iteration's chunk-0 projections, and the final
pair's context eviction/normalize is split in column halves (with the last
two diagonal blocks routed to a second PSUM accumulator) so the output
projection tail overlaps them. All input DMAs ride the SP queue, keeping
the ACT sequencer free for exp issue.
"""

import numpy as np

import concourse.bacc as bacc
import concourse.mybir as mybir
import concourse.tile as tile
from concourse import bass_utils
import concourse.bass as bass_mod

F32 = mybir.dt.float32
F32R = mybir.dt.float32r
BF16 = mybir.dt.bfloat16

B, S, D = 2, 2048, 1024
H, HD = 16, 64
NCORES = 8
HPC = 4            # heads per core
DC = HPC * HD      # 256 D-columns per core
NPAIR = 2          # head pairs per core (128 partitions each)
QSB = 512          # q superblock
KB = 128           # k block
NKT = S // KB      # 16 k tiles
NCH = S // QSB     # 4 chunks / q superblocks
KT = D // 128      # 8 contraction k-tiles for projections


def bcast_pair(t):
    """[128, N] tile viewed as [128, 2, N] with 0-stride middle dim."""
    return bass_mod.AP(tensor=t.tensor, offset=t.offset,
                       ap=[t.ap[0], [0, 2], t.ap[1]])


def build(loop_n=None):
    nc = bacc.Bacc("TRN2", target_bir_lowering=False, debug=False)

    xt_h = nc.dram_tensor("xt", [D, S], F32R, kind="ExternalInput")
    wq = nc.dram_tensor("wq", [D, DC], F32R, kind="ExternalInput")
    wk = nc.dram_tensor("wk", [D, DC], F32R, kind="ExternalInput")
    wv = nc.dram_tensor("wv", [D, DC], F32R, kind="ExternalInput")
    wo = nc.dram_tensor("wo", [DC, D], F32R, kind="ExternalInput")
    bq = nc.dram_tensor("bq", [DC], F32, kind="ExternalInput")
    bk = nc.dram_tensor("bk", [DC], F32, kind="ExternalInput")
    bv = nc.dram_tensor("bv", [DC], F32R, kind="ExternalInput")
    out = nc.dram_tensor("out", [S, D], F32, kind="ExternalOutput")

    with tile.TileContext(nc) as tc:
        with tc.tile_pool(name="persist", bufs=1) as pp, \
             tc.tile_pool(name="wpool", bufs=1) as wp:
            # ---- constants ----
            scratch = pp.tile([128, 128], F32)
            nc.vector.memset(scratch, 1.0)
            ones_r = pp.tile([128, 512], F32R)
            nc.vector.tensor_copy(
                ones_r.rearrange("p (a b) -> p a b", a=4),
                bass_mod.AP(tensor=scratch.tensor, offset=scratch.offset,
                            ap=[scratch.ap[0], [0, 4], scratch.ap[1]]))
            ones_col65 = pp.tile([65, 64], F32R)
            nc.vector.tensor_copy(ones_col65[64:65, :], scratch[0:1, 0:64])
            ones_col = ones_col65[64:65, :]   # base partition 64, matches sums row

            # triangular stripe mask: keep q >= k (incl diagonal)
            tri_f = pp.tile([128, 128], F32)
            nc.gpsimd.memset(tri_f, 1.0)
            nc.gpsimd.affine_select(
                out=tri_f, in_=tri_f,
                compare_op=mybir.AluOpType.is_ge, fill=0.0,
                base=0, pattern=[[1, 128]], channel_multiplier=-1)
            tri = pp.tile([128, 128], F32R)
            nc.vector.tensor_copy(tri, tri_f)
            # r=3 mask over a 256-wide window: cols 0..127 dead, 128..255 tri
            tri2_f = pp.tile([128, 256], F32)
            nc.gpsimd.memset(tri2_f, 1.0)
            nc.gpsimd.affine_select(
                out=tri2_f, in_=tri2_f,
                compare_op=mybir.AluOpType.is_ge, fill=0.0,
                base=-128, pattern=[[1, 256]], channel_multiplier=-1)
            tri2 = pp.tile([128, 256], F32R)
            nc.vector.tensor_copy(tri2, tri2_f)

            # ---- weights / biases (SP queue; order = arrival order) ----
            wq_sb = wp.tile([128, KT, DC], F32R)
            wk_sb = wp.tile([128, KT, DC], F32R)
            wv_sb = wp.tile([128, KT, DC], F32R)
            bq_sb = pp.tile([128, NPAIR], F32)
            bk_sb = pp.tile([128, NPAIR], F32)
            bv_row = pp.tile([1, DC], F32R)
            wo_sb = pp.tile([128, NPAIR, D], F32R)

            # ---- persistent activations ----
            qT = pp.tile([128, NPAIR, S], F32R)       # [hd-pair part, pair, tok]
            kT = pp.tile([128, NPAIR, S], F32R)
            v_all = pp.tile([128, NKT, HPC, 65], F32R)  # [tk part, tktile, head, hd|1]
            ctxT = pp.tile([128, NPAIR, S], F32R)
            bv_rep = pp.tile([128, DC], F32)

            # ones columns of v_all (col 64 of every (tktile, head) slot)
            nc.vector.tensor_copy(
                v_all[:, :, :, 64].rearrange("p a b -> p (a b)"),
                scratch[:, 0:NKT * HPC])

            warm_sb = pp.tile([128, 128], F32)

            with tc.tile_pool(name="xtp", bufs=3) as xt_p, \
                 tc.tile_pool(name="expp", bufs=3) as expp, \
                 tc.tile_pool(name="rec", bufs=6) as rec_p, \
                 tc.tile_pool(name="outp", bufs=2) as out_p, \
                 tc.tile_pool(name="ps_u", bufs=2, space="PSUM") as ps_u, \
                 tc.tile_pool(name="ps_s", bufs=2, space="PSUM") as ps_s, \
                 tc.tile_pool(name="ps_c", bufs=2, space="PSUM") as ps_c:

                xt_tiles = {}

                def dma_x(c):
                    t = xt_p.tile([128, KT, QSB], F32R, name="xt")
                    nc.sync.dma_start(
                        out=t,
                        in_=xt_h.ap()[:, c * QSB:(c + 1) * QSB].rearrange(
                            "(t p) s -> p t s", p=128))
                    xt_tiles[c] = t

                # startup DMAs, all on the SP queue: tiny biases first, then
                # ordered so the p1(0) chain [q0,q1,k0,k1,bv_rep,v*] never
                # stalls mid-flight.
                nc.sync.dma_start(out=bq_sb, in_=bq.ap().rearrange("(t p) -> p t", p=128))
                nc.sync.dma_start(out=bk_sb, in_=bk.ap().rearrange("(t p) -> p t", p=128))
                nc.sync.dma_start(out=bv_row, in_=bv.ap().rearrange("(p n) -> p n", p=1))
                nc.sync.dma_start(out=wq_sb, in_=wq.ap().rearrange("(t p) n -> p t n", p=128))
                dma_x(0)
                nc.sync.dma_start(out=wk_sb, in_=wk.ap().rearrange("(t p) n -> p t n", p=128))
                nc.sync.dma_start(out=wv_sb, in_=wv.ap().rearrange("(t p) n -> p t n", p=128))
                dma_x(1)
                nc.sync.dma_start(out=wo_sb, in_=wo.ap().rearrange("(t p) n -> p t n", p=128))

                # PE warm-up: keeps the clock ramped while startup DMAs land.
                pwarm = ps_u.tile([128, 512], F32, name="pu")
                NWARM = 26
                for w in range(NWARM):
                    nc.tensor.matmul(pwarm, ones_r[:, 0:128], ones_r,
                                     start=(w == 0), stop=(w == NWARM - 1))
                nc.vector.tensor_copy(warm_sb, pwarm[:, 0:128])

                def build_bv_rep():
                    # bv replicated across partitions (once): K=1 ones matmul
                    pbv = ps_u.tile([128, 512], F32, name="pu")
                    nc.tensor.matmul(pbv[:, 0:DC], ones_r[0:1, 0:128], bv_row,
                                     start=True, stop=True)
                    nc.vector.tensor_copy(bv_rep, pbv[:, 0:DC])

                def p1_qk(c, p, w_sb, b_sb, dstT):
                    xt = xt_tiles[c]
                    pq = ps_u.tile([128, 512], F32, name="pu")
                    for kt in range(KT):
                        nc.tensor.matmul(
                            pq, w_sb[:, kt, p * 128:(p + 1) * 128],
                            xt[:, kt, :],
                            start=(kt == 0), stop=(kt == KT - 1))
                    nc.vector.tensor_scalar_add(
                        dstT[:, p, c * QSB:(c + 1) * QSB], pq,
                        b_sb[:, p:p + 1])

                def p1_v(c, tb):
                    xt = xt_tiles[c]
                    pv = ps_u.tile([128, 512], F32, name="pu")
                    for kt in range(KT):
                        nc.tensor.matmul(
                            pv[:, 0:DC],
                            xt[:, kt, tb * 128:(tb + 1) * 128],
                            wv_sb[:, kt, :],
                            start=(kt == 0), stop=(kt == KT - 1))
                    nc.vector.tensor_tensor(
                        v_all[:, 4 * c + tb, :, 0:64],
                        pv[:, 0:DC].rearrange("p (h d) -> p h d", h=HPC),
                        bv_rep.rearrange("p (h d) -> p h d", h=HPC),
                        op=mybir.AluOpType.add)

                def p1_units(c, part):
                    """projections for tokens [512c, 512c+512).

                    part 0: q/k pair0 + v blocks; part 1: q/k pair1;
                    part 2 (startup): everything, ordered by DMA arrival."""
                    if part == 2:
                        for (w_sb, b_sb, dstT) in ((wq_sb, bq_sb, qT),
                                                   (wk_sb, bk_sb, kT)):
                            for p in range(NPAIR):
                                p1_qk(c, p, w_sb, b_sb, dstT)
                                yield
                        build_bv_rep()
                        for tb in range(4):
                            p1_v(c, tb)
                            yield
                        return
                    if part == 0:
                        p1_qk(c, 0, wq_sb, bq_sb, qT)
                        yield
                        p1_qk(c, 0, wk_sb, bk_sb, kT)
                        yield
                        for tb in range(4):  # v natural per 128-token block
                            p1_v(c, tb)
                            yield
                    else:
                        p1_qk(c, 1, wq_sb, bq_sb, qT)
                        yield
                        p1_qk(c, 1, wk_sb, bk_sb, kT)
                        yield

                def p2_units(i):
                    """attention for tq in [512i, 512i+512); yields per (pair, j)."""
                    nj = 4 * i + 4

                    def emit_sxm(p, j):
                        """scores + exp + mask for one (pair, j); returns
                        (ex, lo_col)."""
                        r = j - 4 * i
                        if r < 0:
                            lo_col = 0
                        elif r < 3:
                            lo_col = 128 * r
                        else:
                            lo_col = 256  # r=3 widened to dodge 4x penalty
                        ps = ps_s.tile([128, 2, QSB], F32, name="ps")
                        for hp in range(2):  # head in pair: PE row groups
                            lo = hp * 64
                            nc.tensor.matmul(
                                ps[:, hp, lo_col:],
                                kT[lo:lo + 64, p, j * KB:(j + 1) * KB],
                                qT[lo:lo + 64, p, i * QSB + lo_col:(i + 1) * QSB],
                                start=True, stop=True,
                                tile_position=(lo, 0))
                        ex = expp.tile([128, 2, QSB], F32R, name="ex")
                        nc.scalar.activation(
                            ex[:, :, lo_col:], ps[:, :, lo_col:],
                            mybir.ActivationFunctionType.Exp)
                        if r >= 0:  # diagonal band: mask
                            if r < 3:
                                st = ex[:, :, lo_col:lo_col + 128]
                                m = tri
                            else:
                                st = ex[:, :, lo_col:]
                                m = tri2
                            nc.vector.tensor_tensor(
                                st, st, bcast_pair(m),
                                op=mybir.AluOpType.mult)
                        return ex, lo_col

                    def split_pair(p, pctx):
                        """Last pair: close the main ctx accumulation two
                        blocks early, route the final diagonal blocks into a
                        fresh accumulator, and normalize + project in column
                        halves so the tail chain overlaps the last blocks."""
                        cu2 = []
                        pctx2 = []

                        def norm_half(h):
                            c0, c1 = h * 256, (h + 1) * 256
                            for hp in range(2):
                                eng = nc.gpsimd if hp == 0 else nc.vector
                                cu = cu2[hp]
                                pbc = ps_u.tile([128, 512], F32, name="pu")
                                nc.tensor.matmul(pbc[0:64, 0:256], ones_col,
                                                 cu[64:65, c0:c1],
                                                 start=True, stop=True)
                                rec = rec_p.tile([64, 256], F32, name="rec")
                                nc.vector.reciprocal(rec, pbc[0:64, 0:256])
                                eng.tensor_mul(
                                    ctxT[hp * 64:hp * 64 + 64, p,
                                         i * QSB + c0:i * QSB + c1],
                                    cu[0:64, c0:c1], rec)

                        for j in range(nj):
                            r = j - 4 * i
                            ex, lo_col = emit_sxm(p, j)
                            for hp in range(2):
                                if r < 2:
                                    nc.tensor.matmul(
                                        pctx[hp][:, lo_col:],
                                        v_all[:, j, 2 * p + hp, :],
                                        ex[:, hp, lo_col:],
                                        start=(j == 0), stop=(j == nj - 3))
                                else:
                                    nc.tensor.matmul(
                                        pctx2[hp][:, 256:],
                                        v_all[:, j, 2 * p + hp, :],
                                        ex[:, hp, 256:],
                                        start=(r == 2), stop=(r == 3))
                            yield ("j", p, j, nj)
                            if j == nj - 3:
                                # main group closed: evict full width, then
                                # the final diag blocks get a fresh bank.
                                # (Pool cannot touch PSUM: ACT + DVE.)
                                for hp in range(2):
                                    cu = rec_p.tile([65, QSB], F32R, name="cu")
                                    if hp == 0:
                                        nc.scalar.copy(cu, pctx[hp])
                                    else:
                                        nc.vector.tensor_copy(cu, pctx[hp])
                                    cu2.append(cu)
                                pctx2.extend(
                                    ps_c.tile([65, QSB], F32, name="pctx")
                                    for _ in range(2))
                            if j == nj - 2:
                                norm_half(0)
                                yield ("norm", p, 0)
                        # fold the diag accumulator into cu, then right half
                        for hp in range(2):
                            sl = cu2[hp][:, 256:]
                            nc.vector.tensor_tensor(sl, sl, pctx2[hp][:, 256:],
                                                    op=mybir.AluOpType.add)
                        norm_half(1)
                        yield ("norm", p, 1)

                    for p in range(NPAIR):
                        split = (i == NCH - 1 and p == NPAIR - 1)
                        pctx = [ps_c.tile([65, QSB], F32, name="pctx")
                                for _ in range(2)]
                        if split:
                            yield from split_pair(p, pctx)
                        else:
                            for j in range(nj):
                                ex, lo_col = emit_sxm(p, j)
                                for hp in range(2):
                                    nc.tensor.matmul(
                                        pctx[hp][:, lo_col:],
                                        v_all[:, j, 2 * p + hp, :],
                                        ex[:, hp, lo_col:],
                                        start=(j == 0), stop=(j == nj - 1))
                                yield ("j", p, j, nj)
                            # evict raw pctx (Pool), then normalize via the
                            # K=1 broadcast matmul + reciprocal + Pool mul.
                            work = []
                            for hp in range(2):
                                cu = rec_p.tile([65, QSB], F32R, name="cu")
                                if hp == 0:
                                    nc.scalar.copy(cu, pctx[hp])
                                else:
                                    nc.vector.tensor_copy(cu, pctx[hp])
                                sums = cu[64:65, :]
                                pbc = ps_c.tile([65, QSB], F32, name="pctx")
                                nc.tensor.matmul(pbc[0:64, :], ones_col, sums,
                                                 start=True, stop=True)
                                rec = rec_p.tile([64, QSB], F32, name="rec")
                                nc.vector.reciprocal(rec, pbc[0:64, :])
                                work.append((cu, rec, hp * 64))
                            for (cu, rec, lo) in work:
                                nc.gpsimd.tensor_mul(
                                    ctxT[lo:lo + 64, p, i * QSB:(i + 1) * QSB],
                                    cu[0:64, :], rec)
                            yield ("norm", p, 0)

                def p3_units(i, tpairs=(0, 1), alt=False):
                    """output projection for tq in [512i, 512i+512)."""
                    for tpair in tpairs:
                        ob = out_p.tile([128, 2, D], F32, name="ob")
                        for t2 in range(2):
                            tt = 4 * i + 2 * tpair + t2
                            for d in range(2):
                                po = ps_u.tile([128, 512], F32, name="pu")
                                for p in range(NPAIR):
                                    nc.tensor.matmul(
                                        po, ctxT[:, p, tt * 128:(tt + 1) * 128],
                                        wo_sb[:, p, d * QSB:(d + 1) * QSB],
                                        start=(p == 0), stop=(p == NPAIR - 1))
                                if alt and d == 1:
                                    nc.scalar.copy(
                                        ob[:, t2, d * QSB:(d + 1) * QSB], po)
                                else:
                                    nc.vector.tensor_copy(
                                        ob[:, t2, d * QSB:(d + 1) * QSB], po)
                                yield
                        tt0 = 4 * i + 2 * tpair
                        if i == NCH - 1:
                            # tail: small stores, issued as soon as each
                            # half-row is ready
                            for t2 in range(2):
                                for d in range(2):
                                    nc.sync.dma_start(
                                        out=out.ap()[(tt0 + t2) * 128:
                                                     (tt0 + t2 + 1) * 128,
                                                     d * QSB:(d + 1) * QSB],
                                        in_=ob[:, t2, d * QSB:(d + 1) * QSB])
                        else:
                            nc.sync.dma_start(
                                out=out.ap()[tt0 * 128:(tt0 + 2) * 128, :].rearrange(
                                    "(t p) d -> p t d", p=128),
                                in_=ob)

                # interleaved emission: phase1(c+1) + phase3(c-1) ride along
                # phase2(c)'s j-loop so PE fills ACT-wait gaps.
                import contextlib
                loop_cm = tc.For_i(0, loop_n, 1) if loop_n else contextlib.nullcontext()
                # Software-pipelined across the hardware loop: the body
                # ends by projecting chunk 0/1 of the NEXT iteration during
                # p2(3)'s ACT-paced stretch (their last upstream readers are
                # p2(3) pair-1 j<=3); a one-time prologue primes iteration 0.
                for _ in p1_units(0, 2):
                    pass
                with loop_cm:
                  for c in range(NCH):
                      if c + 2 < NCH:
                          dma_x(c + 2)
                      if c == NCH - 2:
                          dma_x(0)   # next iteration's chunks
                          dma_x(1)
                      extras = []
                      if c + 1 < NCH:
                          extras.append(p1_units(c + 1, 0))
                          extras.append(p1_units(c + 1, 1))
                      if 0 < c < NCH - 1:
                          extras.append(p3_units(c - 1))

                      def drain_extras(k):
                          n = 0
                          while extras and n < k:
                              try:
                                  next(extras[0])
                                  n += 1
                              except StopIteration:
                                  extras.pop(0)

                      nj = 4 * c + 4
                      if c == NCH - 1:
                          # p3(c-1) paced over pair 0; next iteration's p1(0)
                          # over pair 1 (after j=3); p3(c) split by token-pair
                          # and hooked after each final norm half.
                          extras.append(p3_units(c - 1))
                          gen = p2_units(c)
                          for k in range(nj + 1):
                              next(gen)          # pair-0: j blocks + norm
                              if k % 2 == 1:
                                  drain_extras(1)
                          drain_extras(1000)
                          extras.append(p1_units(0, 0))
                          extras.append(p1_units(0, 1))
                          for k in range(nj - 1):
                              next(gen)          # pair-1: j = 0..nj-2
                              if k >= 4:
                                  drain_extras(1)
                          drain_extras(1000)
                          next(gen)              # norm half 0 (left columns)
                          next(gen)              # last j block
                          for _ in p3_units(c, tpairs=(0,), alt=True):
                              pass
                          next(gen)              # norm half 1 (right columns)
                          for _ in p3_units(c, tpairs=(1,), alt=True):
                              pass
                          for _ in gen:
                              pass
                      else:
                          nx = 8 + (8 if 0 < c < NCH - 1 else 0)
                          per = max(1, -(-nx // (2 * (nj + 1))))
                          for tag in p2_units(c):
                              if tag[0] == "j" and tag[2] >= tag[3] - 2:
                                  continue  # hold extras for the pair boundary
                              drain_extras(per if tag[0] == "j" else 3)
                          drain_extras(1000)

    nc.compile()
    return nc


def _bf16(a):
    import ml_dtypes
    return np.ascontiguousarray(a.astype(ml_dtypes.bfloat16))


def prepare_in_maps(x, Wq, bq_, Wk, bk_, Wv, bv_, Wo, bo_):
    x = np.asarray(x, np.float32)
    xtc = [np.ascontiguousarray(x[0].T), np.ascontiguousarray(x[1].T)]
    in_maps = []
    for c in range(NCORES):
        b = c // 4
        g = c % 4
        sl = slice(DC * g, DC * (g + 1))
        in_maps.append({
            "xt": xtc[b],
            "wq": np.ascontiguousarray(np.asarray(Wq, np.float32)[:, sl] * 0.125),
            "wk": np.ascontiguousarray(np.asarray(Wk, np.float32)[:, sl]),
            "wv": np.ascontiguousarray(np.asarray(Wv, np.float32)[:, sl]),
            "wo": np.ascontiguousarray(np.asarray(Wo, np.float32)[sl, :]),
            "bq": np.ascontiguousarray(np.asarray(bq_, np.float32)[sl] * 0.125),
            "bk": np.ascontiguousarray(np.asarray(bk_, np.float32)[sl]),
            "bv": np.ascontiguousarray(np.asarray(bv_, np.float32)[sl]),
        })
    return in_maps


_NC_CACHE = {}


def _get_nc():
    if "nc" not in _NC_CACHE:
        _NC_CACHE["nc"] = build()
    return _NC_CACHE["nc"]


def kernel(x, Wq, bq, Wk, bk, Wv, bv, Wo, bo, _trace=False):
    nc = _get_nc()
    in_maps = prepare_in_maps(x, Wq, bq, Wk, bk, Wv, bv, Wo, bo)
    res = bass_utils.run_bass_kernel_spmd(
        nc, in_maps, core_ids=list(range(NCORES)), trace=_trace)
    if _trace:
        _NC_CACHE["last_results"] = res
    partials = [res.results[c]["out"] for c in range(NCORES)]
    bo = np.asarray(bo, np.float32)
    full = np.stack([
        partials[0] + partials[1] + partials[2] + partials[3] + bo,
        partials[4] + partials[5] + partials[6] + partials[7] + bo,
    ]).astype(np.float32)
    return full


# revision 51
# speedup vs baseline: 1.4402x; 1.4402x over previous
"""Trainium2 Bass kernel for 16-head causal MHA (B=2, S=2048, D=1024), fp32.

Sharding (8 cores): batch x head-group. Core c handles batch c//4 and heads
4*(c%4)..4*(c%4)+3 (D columns 256*(c%4) .. +256). QKV weights column-sharded,
Wo row-sharded (Megatron). Per-core partial outputs are summed on the host
(the row-parallel all-reduce), bo added once per batch.

x is transposed on the HOST (xt = x[b].T, [D, S]) so the kernel needs no PE
transposes at all: projections read xt directly as [D-part, tok] tiles.

Per-core dataflow (all matmuls in float32r: ~12-bit mantissa, full speed):
  phase 1 (per 512-token chunk): qT/kT = W.T @ xt ([head-dim part, tok free])
           + bias; v computed in NATURAL layout per 128-token block
           (xt-block stationary, Wv moving: out [tok part, 256]), written
           into v_all slots with a ones column appended (softmax denom).
  phase 2 (per q-superblock i, head-pair p, k-block j<=4i+3):
           S_T[tk,tq] = k @ qT (2 heads packed in PE row groups);
           expS = exp(S_T) straight to f32r (no max subtraction; scores O(6));
           diagonal-band tiles: triangular mask-mul (DVE); the last diag
           block (r=3) runs at free=256 (not 128) to dodge the fp32r
           small-free 4x penalty, with a [0|tri] mask.
           ctxT[hd+1, tq] += [v|1].T @ expS  (ones row = softmax denominator);
           evict via ACT/DVE copies (GPSIMD cannot touch PSUM), normalize
           via K=1 broadcast matmul + DVE reciprocal + Pool mul.
  phase 3: out[tq, :] = ctxT_norm.T @ Wo_slice (partial), DMA out (SP queue,
           HW DGE; Pool's software DGE costs ~1us of Pool time per store).

Emission interleaves phase1(c+1) / phase3(c-1) units into phase2(c)'s
j-loop so PE fills its ACT-wait gaps with projection work, and is
software-pipelined across the loop_n hardware loop: the body's last
superblock carries the NEXT iteration's chunk-0 projections, and the final
pair's context eviction/normalize is split in column halves (with the last
two diagonal blocks routed to a second PSUM accumulator) so the output
projection tail overlaps them. All input DMAs ride the SP queue, keeping
the ACT sequencer free for exp issue.
"""

import numpy as np

import concourse.bacc as bacc
import concourse.mybir as mybir
import concourse.tile as tile
from concourse import bass_utils
import concourse.bass as bass_mod

F32 = mybir.dt.float32
F32R = mybir.dt.float32r
BF16 = mybir.dt.bfloat16

B, S, D = 2, 2048, 1024
H, HD = 16, 64
NCORES = 8
HPC = 4            # heads per core
DC = HPC * HD      # 256 D-columns per core
NPAIR = 2          # head pairs per core (128 partitions each)
QSB = 512          # q superblock
KB = 128           # k block
NKT = S // KB      # 16 k tiles
NCH = S // QSB     # 4 chunks / q superblocks
KT = D // 128      # 8 contraction k-tiles for projections


def bcast_pair(t):
    """[128, N] tile viewed as [128, 2, N] with 0-stride middle dim."""
    return bass_mod.AP(tensor=t.tensor, offset=t.offset,
                       ap=[t.ap[0], [0, 2], t.ap[1]])


def build(loop_n=None):
    nc = bacc.Bacc("TRN2", target_bir_lowering=False, debug=False)

    xt_h = nc.dram_tensor("xt", [D, S], F32R, kind="ExternalInput")
    wq = nc.dram_tensor("wq", [D, DC], F32R, kind="ExternalInput")
    wk = nc.dram_tensor("wk", [D, DC], F32R, kind="ExternalInput")
    wv = nc.dram_tensor("wv", [D, DC], F32R, kind="ExternalInput")
    wo = nc.dram_tensor("wo", [DC, D], F32R, kind="ExternalInput")
    bq = nc.dram_tensor("bq", [DC], F32, kind="ExternalInput")
    bk = nc.dram_tensor("bk", [DC], F32, kind="ExternalInput")
    bv = nc.dram_tensor("bv", [DC], F32R, kind="ExternalInput")
    out = nc.dram_tensor("out", [S, D], F32, kind="ExternalOutput")

    with tile.TileContext(nc) as tc:
        with tc.tile_pool(name="persist", bufs=1) as pp, \
             tc.tile_pool(name="wpool", bufs=1) as wp:
            # ---- constants ----
            scratch = pp.tile([128, 128], F32)
            nc.vector.memset(scratch, 1.0)
            ones_r = pp.tile([128, 512], F32R)
            nc.vector.tensor_copy(
                ones_r.rearrange("p (a b) -> p a b", a=4),
                bass_mod.AP(tensor=scratch.tensor, offset=scratch.offset,
                            ap=[scratch.ap[0], [0, 4], scratch.ap[1]]))
            ones_col65 = pp.tile([65, 64], F32R)
            nc.vector.tensor_copy(ones_col65[64:65, :], scratch[0:1, 0:64])
            ones_col = ones_col65[64:65, :]   # base partition 64, matches sums row

            # triangular stripe mask: keep q >= k (incl diagonal)
            tri_f = pp.tile([128, 128], F32)
            nc.gpsimd.memset(tri_f, 1.0)
            nc.gpsimd.affine_select(
                out=tri_f, in_=tri_f,
                compare_op=mybir.AluOpType.is_ge, fill=0.0,
                base=0, pattern=[[1, 128]], channel_multiplier=-1)
            tri = pp.tile([128, 128], F32R)
            nc.vector.tensor_copy(tri, tri_f)
            # r=3 mask over a 256-wide window: cols 0..127 dead, 128..255 tri
            tri2_f = pp.tile([128, 256], F32)
            nc.gpsimd.memset(tri2_f, 1.0)
            nc.gpsimd.affine_select(
                out=tri2_f, in_=tri2_f,
                compare_op=mybir.AluOpType.is_ge, fill=0.0,
                base=-128, pattern=[[1, 256]], channel_multiplier=-1)
            tri2 = pp.tile([128, 256], F32R)
            nc.vector.tensor_copy(tri2, tri2_f)

            # ---- weights / biases (SP queue; order = arrival order) ----
            wq_sb = wp.tile([128, KT, DC], F32R)
            wk_sb = wp.tile([128, KT, DC], F32R)
            wv_sb = wp.tile([128, KT, DC], F32R)
            bq_sb = pp.tile([128, NPAIR], F32)
            bk_sb = pp.tile([128, NPAIR], F32)
            bv_row = pp.tile([1, DC], F32R)
            wo_sb = pp.tile([128, NPAIR, D], F32R)

            # ---- persistent activations ----
            qT = pp.tile([128, NPAIR, S], F32R)       # [hd-pair part, pair, tok]
            kT = pp.tile([128, NPAIR, S], F32R)
            v_all = pp.tile([128, NKT, HPC, 65], F32R)  # [tk part, tktile, head, hd|1]
            ctxT = pp.tile([128, NPAIR, S], F32R)
            bv_rep = pp.tile([128, DC], F32)

            # ones columns of v_all (col 64 of every (tktile, head) slot)
            nc.vector.tensor_copy(
                v_all[:, :, :, 64].rearrange("p a b -> p (a b)"),
                scratch[:, 0:NKT * HPC])

            warm_sb = pp.tile([128, 128], F32)

            with tc.tile_pool(name="xtp", bufs=3) as xt_p, \
                 tc.tile_pool(name="expp", bufs=3) as expp, \
                 tc.tile_pool(name="rec", bufs=6) as rec_p, \
                 tc.tile_pool(name="outp", bufs=2) as out_p, \
                 tc.tile_pool(name="ps_u", bufs=2, space="PSUM") as ps_u, \
                 tc.tile_pool(name="ps_s", bufs=2, space="PSUM") as ps_s, \
                 tc.tile_pool(name="ps_c", bufs=2, space="PSUM") as ps_c:

                xt_tiles = {}

                def dma_x(c):
                    t = xt_p.tile([128, KT, QSB], F32R, name="xt")
                    nc.sync.dma_start(
                        out=t,
                        in_=xt_h.ap()[:, c * QSB:(c + 1) * QSB].rearrange(
                            "(t p) s -> p t s", p=128))
                    xt_tiles[c] = t

                # startup DMAs, all on the SP queue: tiny biases first, then
                # ordered so the p1(0) chain [q0,q1,k0,k1,bv_rep,v*] never
                # stalls mid-flight.
                nc.sync.dma_start(out=bq_sb, in_=bq.ap().rearrange("(t p) -> p t", p=128))
                nc.sync.dma_start(out=bk_sb, in_=bk.ap().rearrange("(t p) -> p t", p=128))
                nc.sync.dma_start(out=bv_row, in_=bv.ap().rearrange("(p n) -> p n", p=1))
                nc.sync.dma_start(out=wq_sb, in_=wq.ap().rearrange("(t p) n -> p t n", p=128))
                dma_x(0)
                nc.sync.dma_start(out=wk_sb, in_=wk.ap().rearrange("(t p) n -> p t n", p=128))
                nc.sync.dma_start(out=wv_sb, in_=wv.ap().rearrange("(t p) n -> p t n", p=128))
                dma_x(1)
                nc.sync.dma_start(out=wo_sb, in_=wo.ap().rearrange("(t p) n -> p t n", p=128))

                # PE warm-up: keeps the clock ramped while startup DMAs land.
                pwarm = ps_u.tile([128, 512], F32, name="pu")
                NWARM = 26
                for w in range(NWARM):
                    nc.tensor.matmul(pwarm, ones_r[:, 0:128], ones_r,
                                     start=(w == 0), stop=(w == NWARM - 1))
                nc.vector.tensor_copy(warm_sb, pwarm[:, 0:128])

                def build_bv_rep():
                    # bv replicated across partitions (once): K=1 ones matmul
                    pbv = ps_u.tile([128, 512], F32, name="pu")
                    nc.tensor.matmul(pbv[:, 0:DC], ones_r[0:1, 0:128], bv_row,
                                     start=True, stop=True)
                    nc.vector.tensor_copy(bv_rep, pbv[:, 0:DC])

                def p1_qk(c, p, w_sb, b_sb, dstT):
                    xt = xt_tiles[c]
                    pq = ps_u.tile([128, 512], F32, name="pu")
                    for kt in range(KT):
                        nc.tensor.matmul(
                            pq, w_sb[:, kt, p * 128:(p + 1) * 128],
                            xt[:, kt, :],
                            start=(kt == 0), stop=(kt == KT - 1))
                    nc.vector.tensor_scalar_add(
                        dstT[:, p, c * QSB:(c + 1) * QSB], pq,
                        b_sb[:, p:p + 1])

                def p1_v(c, tb):
                    xt = xt_tiles[c]
                    pv = ps_u.tile([128, 512], F32, name="pu")
                    for kt in range(KT):
                        nc.tensor.matmul(
                            pv[:, 0:DC],
                            xt[:, kt, tb * 128:(tb + 1) * 128],
                            wv_sb[:, kt, :],
                            start=(kt == 0), stop=(kt == KT - 1))
                    nc.vector.tensor_tensor(
                        v_all[:, 4 * c + tb, :, 0:64],
                        pv[:, 0:DC].rearrange("p (h d) -> p h d", h=HPC),
                        bv_rep.rearrange("p (h d) -> p h d", h=HPC),
                        op=mybir.AluOpType.add)

                def p1_units(c, part):
                    """projections for tokens [512c, 512c+512).

                    part 0: q/k pair0 + v blocks; part 1: q/k pair1;
                    part 2 (startup): everything, ordered by DMA arrival."""
                    if part == 2:
                        for (w_sb, b_sb, dstT) in ((wq_sb, bq_sb, qT),
                                                   (wk_sb, bk_sb, kT)):
                            for p in range(NPAIR):
                                p1_qk(c, p, w_sb, b_sb, dstT)
                                yield
                        build_bv_rep()
                        for tb in range(4):
                            p1_v(c, tb)
                            yield
                        return
                    if part == 0:
                        p1_qk(c, 0, wq_sb, bq_sb, qT)
                        yield
                        p1_qk(c, 0, wk_sb, bk_sb, kT)
                        yield
                        for tb in range(4):  # v natural per 128-token block
                            p1_v(c, tb)
                            yield
                    else:
                        p1_qk(c, 1, wq_sb, bq_sb, qT)
                        yield
                        p1_qk(c, 1, wk_sb, bk_sb, kT)
                        yield

                def p2_units(i):
                    """attention for tq in [512i, 512i+512); yields per (pair, j)."""
                    nj = 4 * i + 4

                    def emit_sxm(p, j):
                        """scores + exp + mask for one (pair, j); returns
                        (ex, lo_col)."""
                        r = j - 4 * i
                        if r < 0:
                            lo_col = 0
                        elif r < 3:
                            lo_col = 128 * r
                        else:
                            lo_col = 256  # r=3 widened to dodge 4x penalty
                        ps = ps_s.tile([128, 2, QSB], F32, name="ps")
                        for hp in range(2):  # head in pair: PE row groups
                            lo = hp * 64
                            nc.tensor.matmul(
                                ps[:, hp, lo_col:],
                                kT[lo:lo + 64, p, j * KB:(j + 1) * KB],
                                qT[lo:lo + 64, p, i * QSB + lo_col:(i + 1) * QSB],
                                start=True, stop=True,
                                tile_position=(lo, 0))
                        ex = expp.tile([128, 2, QSB], F32R, name="ex")
                        nc.scalar.activation(
                            ex[:, :, lo_col:], ps[:, :, lo_col:],
                            mybir.ActivationFunctionType.Exp)
                        if r >= 0:  # diagonal band: mask
                            if r < 3:
                                st = ex[:, :, lo_col:lo_col + 128]
                                m = tri
                            else:
                                st = ex[:, :, lo_col:]
                                m = tri2
                            nc.vector.tensor_tensor(
                                st, st, bcast_pair(m),
                                op=mybir.AluOpType.mult)
                        return ex, lo_col

                    def split_pair(p, pctx):
                        """Last pair: close the main ctx accumulation two
                        blocks early, route the final diagonal blocks into a
                        fresh accumulator, and normalize + project in column
                        halves so the tail chain overlaps the last blocks."""
                        cu2 = []
                        pctx2 = []

                        def norm_half(h):
                            c0, c1 = h * 256, (h + 1) * 256
                            for hp in range(2):
                                eng = nc.gpsimd if hp == 0 else nc.vector
                                cu = cu2[hp]
                                pbc = ps_u.tile([128, 512], F32, name="pu")
                                nc.tensor.matmul(pbc[0:64, 0:256], ones_col,
                                                 cu[64:65, c0:c1],
                                                 start=True, stop=True)
                                rec = rec_p.tile([64, 256], F32, name="rec")
                                nc.vector.reciprocal(rec, pbc[0:64, 0:256])
                                eng.tensor_mul(
                                    ctxT[hp * 64:hp * 64 + 64, p,
                                         i * QSB + c0:i * QSB + c1],
                                    cu[0:64, c0:c1], rec)

                        for j in range(nj):
                            r = j - 4 * i
                            ex, lo_col = emit_sxm(p, j)
                            for hp in range(2):
                                if r < 2:
                                    nc.tensor.matmul(
                                        pctx[hp][:, lo_col:],
                                        v_all[:, j, 2 * p + hp, :],
                                        ex[:, hp, lo_col:],
                                        start=(j == 0), stop=(j == nj - 3))
                                else:
                                    nc.tensor.matmul(
                                        pctx2[hp][:, 256:],
                                        v_all[:, j, 2 * p + hp, :],
                                        ex[:, hp, 256:],
                                        start=(r == 2), stop=(r == 3))
                            yield ("j", p, j, nj)
                            if j == nj - 3:
                                # main group closed: evict full width, then
                                # the final diag blocks get a fresh bank.
                                # (Pool cannot touch PSUM: ACT + DVE.)
                                for hp in range(2):
                                    cu = rec_p.tile([65, QSB], F32R, name="cu")
                                    if hp == 0:
                                        nc.scalar.copy(cu, pctx[hp])
                                    else:
                                        nc.vector.tensor_copy(cu, pctx[hp])
                                    cu2.append(cu)
                                pctx2.extend(
                                    ps_c.tile([65, QSB], F32, name="pctx")
                                    for _ in range(2))
                            if j == nj - 2:
                                norm_half(0)
                                yield ("norm", p, 0)
                        # fold the diag accumulator into cu, then right half
                        for hp in range(2):
                            sl = cu2[hp][:, 256:]
                            nc.vector.tensor_tensor(sl, sl, pctx2[hp][:, 256:],
                                                    op=mybir.AluOpType.add)
                        norm_half(1)
                        yield ("norm", p, 1)

                    for p in range(NPAIR):
                        split = (i == NCH - 1 and p == NPAIR - 1)
                        pctx = [ps_c.tile([65, QSB], F32, name="pctx")
                                for _ in range(2)]
                        if split:
                            yield from split_pair(p, pctx)
                        else:
                            for j in range(nj):
                                ex, lo_col = emit_sxm(p, j)
                                for hp in range(2):
                                    nc.tensor.matmul(
                                        pctx[hp][:, lo_col:],
                                        v_all[:, j, 2 * p + hp, :],
                                        ex[:, hp, lo_col:],
                                        start=(j == 0), stop=(j == nj - 1))
                                yield ("j", p, j, nj)
                            # let the driver queue PE extras here: they fill
                            # the eviction window so the bcast matmul below
                            # doesn't head-of-line block the PE queue.
                            yield ("pre", p, 0)
                            # evict raw pctx, then normalize via the K=1
                            # broadcast matmul + reciprocal + Pool mul.
                            work = []
                            for hp in range(2):
                                cu = rec_p.tile([65, QSB], F32R, name="cu")
                                if hp == 0:
                                    nc.scalar.copy(cu, pctx[hp])
                                else:
                                    nc.vector.tensor_copy(cu, pctx[hp])
                                sums = cu[64:65, :]
                                pbc = ps_c.tile([65, QSB], F32, name="pctx")
                                nc.tensor.matmul(pbc[0:64, :], ones_col, sums,
                                                 start=True, stop=True)
                                rec = rec_p.tile([64, QSB], F32, name="rec")
                                nc.vector.reciprocal(rec, pbc[0:64, :])
                                work.append((cu, rec, hp * 64))
                            for (cu, rec, lo) in work:
                                nc.gpsimd.tensor_mul(
                                    ctxT[lo:lo + 64, p, i * QSB:(i + 1) * QSB],
                                    cu[0:64, :], rec)
                            yield ("norm", p, 0)

                def p3_units(i, tpairs=(0, 1), alt=False):
                    """output projection for tq in [512i, 512i+512)."""
                    for tpair in tpairs:
                        ob = out_p.tile([128, 2, D], F32, name="ob")
                        for t2 in range(2):
                            tt = 4 * i + 2 * tpair + t2
                            for d in range(2):
                                po = ps_u.tile([128, 512], F32, name="pu")
                                for p in range(NPAIR):
                                    nc.tensor.matmul(
                                        po, ctxT[:, p, tt * 128:(tt + 1) * 128],
                                        wo_sb[:, p, d * QSB:(d + 1) * QSB],
                                        start=(p == 0), stop=(p == NPAIR - 1))
                                if alt and d == 1:
                                    nc.scalar.copy(
                                        ob[:, t2, d * QSB:(d + 1) * QSB], po)
                                else:
                                    nc.vector.tensor_copy(
                                        ob[:, t2, d * QSB:(d + 1) * QSB], po)
                                yield
                        tt0 = 4 * i + 2 * tpair
                        if i == NCH - 1:
                            # tail: small stores, issued as soon as each
                            # half-row is ready
                            for t2 in range(2):
                                for d in range(2):
                                    nc.sync.dma_start(
                                        out=out.ap()[(tt0 + t2) * 128:
                                                     (tt0 + t2 + 1) * 128,
                                                     d * QSB:(d + 1) * QSB],
                                        in_=ob[:, t2, d * QSB:(d + 1) * QSB])
                        else:
                            nc.sync.dma_start(
                                out=out.ap()[tt0 * 128:(tt0 + 2) * 128, :].rearrange(
                                    "(t p) d -> p t d", p=128),
                                in_=ob)

                # interleaved emission: phase1(c+1) + phase3(c-1) ride along
                # phase2(c)'s j-loop so PE fills ACT-wait gaps.
                import contextlib
                loop_cm = tc.For_i(0, loop_n, 1) if loop_n else contextlib.nullcontext()
                # Software-pipelined across the hardware loop: the body
                # ends by projecting chunk 0/1 of the NEXT iteration during
                # p2(3)'s ACT-paced stretch (their last upstream readers are
                # p2(3) pair-1 j<=3); a one-time prologue primes iteration 0.
                for _ in p1_units(0, 2):
                    pass
                with loop_cm:
                  for c in range(NCH):
                      if c + 2 < NCH:
                          dma_x(c + 2)
                      if c == NCH - 2:
                          dma_x(0)   # next iteration's chunks
                          dma_x(1)
                      extras = []
                      if c + 1 < NCH:
                          extras.append(p1_units(c + 1, 0))
                          extras.append(p1_units(c + 1, 1))
                      if 0 < c < NCH - 1:
                          extras.append(p3_units(c - 1))

                      def drain_extras(k):
                          n = 0
                          while extras and n < k:
                              try:
                                  next(extras[0])
                                  n += 1
                              except StopIteration:
                                  extras.pop(0)

                      nj = 4 * c + 4
                      if c == NCH - 1:
                          # p3(c-1) paced over pair 0; next iteration's p1(0)
                          # over pair 1 (after j=3); p3(c) split by token-pair
                          # and hooked after each final norm half.
                          extras.append(p3_units(c - 1))
                          gen = p2_units(c)
                          for k in range(nj + 2):
                              next(gen)          # pair-0: j, pre-norm, norm
                              if k % 2 == 1:
                                  drain_extras(1)
                          drain_extras(1000)
                          extras.append(p1_units(0, 0))
                          extras.append(p1_units(0, 1))
                          for k in range(nj - 1):
                              next(gen)          # pair-1: j = 0..nj-2
                              if k >= 4:
                                  drain_extras(1)
                          drain_extras(1000)
                          next(gen)              # norm half 0 (left columns)
                          next(gen)              # last j block
                          for _ in p3_units(c, tpairs=(0,), alt=True):
                              pass
                          next(gen)              # norm half 1 (right columns)
                          for _ in p3_units(c, tpairs=(1,), alt=True):
                              pass
                          for _ in gen:
                              pass
                      else:
                          nx = 8 + (8 if 0 < c < NCH - 1 else 0)
                          per = max(1, -(-nx // (2 * (nj + 1))))
                          for tag in p2_units(c):
                              if tag[0] == "j" and tag[2] >= tag[3] - 2:
                                  continue  # hold extras for the pair boundary
                              drain_extras(per if tag[0] == "j" else 3)
                          drain_extras(1000)

    nc.compile()
    return nc


def _bf16(a):
    import ml_dtypes
    return np.ascontiguousarray(a.astype(ml_dtypes.bfloat16))


def prepare_in_maps(x, Wq, bq_, Wk, bk_, Wv, bv_, Wo, bo_):
    x = np.asarray(x, np.float32)
    xtc = [np.ascontiguousarray(x[0].T), np.ascontiguousarray(x[1].T)]
    in_maps = []
    for c in range(NCORES):
        b = c // 4
        g = c % 4
        sl = slice(DC * g, DC * (g + 1))
        in_maps.append({
            "xt": xtc[b],
            "wq": np.ascontiguousarray(np.asarray(Wq, np.float32)[:, sl] * 0.125),
            "wk": np.ascontiguousarray(np.asarray(Wk, np.float32)[:, sl]),
            "wv": np.ascontiguousarray(np.asarray(Wv, np.float32)[:, sl]),
            "wo": np.ascontiguousarray(np.asarray(Wo, np.float32)[sl, :]),
            "bq": np.ascontiguousarray(np.asarray(bq_, np.float32)[sl] * 0.125),
            "bk": np.ascontiguousarray(np.asarray(bk_, np.float32)[sl]),
            "bv": np.ascontiguousarray(np.asarray(bv_, np.float32)[sl]),
        })
    return in_maps


_NC_CACHE = {}


def _get_nc():
    if "nc" not in _NC_CACHE:
        _NC_CACHE["nc"] = build()
    return _NC_CACHE["nc"]


def kernel(x, Wq, bq, Wk, bk, Wv, bv, Wo, bo, _trace=False):
    nc = _get_nc()
    in_maps = prepare_in_maps(x, Wq, bq, Wk, bk, Wv, bv, Wo, bo)
    res = bass_utils.run_bass_kernel_spmd(
        nc, in_maps, core_ids=list(range(NCORES)), trace=_trace)
    if _trace:
        _NC_CACHE["last_results"] = res
    partials = [res.results[c]["out"] for c in range(NCORES)]
    bo = np.asarray(bo, np.float32)
    full = np.stack([
        partials[0] + partials[1] + partials[2] + partials[3] + bo,
        partials[4] + partials[5] + partials[6] + partials[7] + bo,
    ]).astype(np.float32)
    return full
